# revision 1
# baseline (speedup 1.0000x reference)
"""Trainium2 Bass kernel for nn_BatchProgramCC (gnn_message_passing).

Pipeline (2 NEFF launches):
  Host:  TW = emb @ Wc.T + bc  (weight-only constant fold), cast bf16.
  K1 (8 cores, tree-sharded): batched indirect-DMA gather of TW rows by
      token (8 instrs x 8192 rows), per-tree subtree sums via bf16
      structure matmuls (output transposed to [ch, node]), per-tree max
      on DVE (4 trees per reduce from 2-bank PSUM), relu -> te shard.
  K2 (2 cores: fwd / bwd): parallel-in-time GRU via DEER fixed-point
      iteration.  Given gates, the h-recurrence is linear-diagonal:
      h_t = z_t*h_{t-1} + (1-z_t)*n_t, evaluated with the hardware
      tensor_tensor_scan.  Gates are recomputed from the previous
      iterate's h-sequence with batched matmuls.  6 iterations converge
      to ~3e-4 (fp32 numpy) vs the 2e-2 gate.  Final max over t on DVE.

Self-contained: hardcodes all shapes; no sibling imports.
"""

import numpy as np
import ml_dtypes

import concourse.bass as bass
import concourse.mybir as mybir
from concourse import bacc
from concourse.tile import TileContext
from concourse.bass_utils import run_bass_kernel_spmd

F32 = mybir.dt.float32
BF16 = mybir.dt.bfloat16
I16 = mybir.dt.int16
I32 = mybir.dt.int32

T_TREES = 2048
P = 256          # nodes per tree
KARY = 4
VOCAB = 30000
E = 128
C = 128
H = 128
NCORES = 8
TREES_PER_CORE = T_TREES // NCORES          # 256
NODES_PER_CORE = TREES_PER_CORE * P         # 65536

DEER_ITERS = 6

LAST_RESULTS = []   # BassKernelResults stash for test.py profiling
_TRACE_KW = {}      # test.py may set {'trace': True}


def _tree_struct():
    """S[i, j] = 1 iff node j is in subtree(i) (including i==j)."""
    pl = np.zeros(P, np.int64)
    for i in range(1, P):
        pl[i] = (i - 1) // KARY
    S = np.zeros((P, P), np.float32)
    for j in range(P):
        a = j
        while True:
            S[a, j] = 1.0
            if a == 0:
                break
            a = int(pl[a])
    return S


# ---------------------------------------------------------------- K1: trees
def build_k1():
    S = _tree_struct()
    nc = bacc.Bacc("TRN2", target_bir_lowering=False, debug=False,
                   num_devices=NCORES)
    tw = nc.dram_tensor("tw", [VOCAB, C], BF16, kind="ExternalInput")
    idx = nc.dram_tensor("idx", [128, NODES_PER_CORE // 16], I16,
                         kind="ExternalInput")
    s00t = nc.dram_tensor("s00t", [128, 128], BF16, kind="ExternalInput")
    rhi = nc.dram_tensor("rhi", [128, 256], BF16, kind="ExternalInput")
    te = nc.dram_tensor("te", [128, TREES_PER_CORE], F32,
                        kind="ExternalOutput")

    NIDX = NODES_PER_CORE // 16      # 4096 idx columns (16-partition wrap)
    GROWS = 8192                     # rows per dma_gather
    NGATHER = NODES_PER_CORE // GROWS   # 8
    GIDXC = GROWS // 16              # 512 idx columns per gather
    TREES_PER_GATHER = GROWS // P    # 32

    with TileContext(nc) as tc:
        with (
            tc.tile_pool(name="const", bufs=1) as cp,
            tc.tile_pool(name="gat", bufs=2) as gp,
            tc.tile_pool(name="psum", bufs=3, space="PSUM") as pp,
        ):
            idx_sb = cp.tile([128, NIDX], I16)
            nc.sync.dma_start(out=idx_sb[:], in_=idx[:])
            s00t_sb = cp.tile([128, 128], BF16)
            nc.sync.dma_start(out=s00t_sb[:], in_=s00t[:])
            rhi_sb = cp.tile([128, 256], BF16)
            nc.sync.dma_start(out=rhi_sb[:], in_=rhi[:])
            te_sb = cp.tile([128, TREES_PER_CORE], F32)

            for g in range(NGATHER):
                gat = gp.tile([128, GROWS // 128, C], BF16, tag="gat")
                nc.gpsimd.dma_gather(
                    gat[:], tw[:],
                    idx_sb[:, g * GIDXC:(g + 1) * GIDXC],
                    GROWS, GROWS, C, single_packet=False)
                # 32 trees; 4 trees share one 2-bank psum tile
                for q in range(TREES_PER_GATHER // 4):
                    ps = pp.tile([128, 4, 256], F32, tag="ps")
                    for ti in range(4):
                        t_in_tile = q * 4 + ti
                        lo = gat[:, 2 * t_in_tile, :]
                        hi = gat[:, 2 * t_in_tile + 1, :]
                        nc.tensor.matmul(out=ps[:, ti, :], lhsT=hi,
                                         rhs=rhi_sb[:], start=True,
                                         stop=False)
                        nc.tensor.matmul(out=ps[:, ti, 0:128], lhsT=lo,
                                         rhs=s00t_sb[:], start=False,
                                         stop=True)
                    t0 = g * TREES_PER_GATHER + q * 4
                    nc.vector.tensor_reduce(
                        out=te_sb[:, t0:t0 + 4], in_=ps[:],
                        axis=mybir.AxisListType.X, op=mybir.AluOpType.max)
            nc.vector.tensor_scalar_max(out=te_sb[:], in0=te_sb[:],
                                        scalar1=0.0)
            nc.sync.dma_start(out=te[:], in_=te_sb[:])
    nc.finalize()
    return nc, S


# ---------------------------------------------------------------- K2: GRU
def build_k2(iters=None):
    iters = iters or DEER_ITERS
    T = T_TREES
    nc = bacc.Bacc("TRN2", target_bir_lowering=False, debug=False,
                   num_devices=2)
    x = nc.dram_tensor("x", [128, T], F32, kind="ExternalInput")
    wiT = nc.dram_tensor("wiT", [128, 384], F32, kind="ExternalInput")
    whT = nc.dram_tensor("whT", [128, 384], BF16, kind="ExternalInput")
    ident = nc.dram_tensor("ident", [128, 128], BF16, kind="ExternalInput")
    gxb = nc.dram_tensor("gxb", [128, 3], F32, kind="ExternalInput")
    bhn = nc.dram_tensor("bhn", [128, 1], F32, kind="ExternalInput")
    hmax = nc.dram_tensor("hmax", [128, 1], F32, kind="ExternalOutput")

    SIG = mybir.ActivationFunctionType.Sigmoid
    TANH = mybir.ActivationFunctionType.Tanh
    MULT = mybir.AluOpType.mult
    ADD = mybir.AluOpType.add
    SUB = mybir.AluOpType.subtract

    CH = 512                 # column chunk
    NCH = T // CH            # 4

    with TileContext(nc) as tc:
        with (
            tc.tile_pool(name="const", bufs=1) as cp,
            tc.tile_pool(name="step", bufs=3) as sp,
            tc.tile_pool(name="psum", bufs=2, space="PSUM") as pp,
        ):
            x_sb = cp.tile([128, T], F32)
            nc.sync.dma_start(out=x_sb[:], in_=x[:])
            wiT_sb = cp.tile([128, 384], F32)
            nc.sync.dma_start(out=wiT_sb[:], in_=wiT[:])
            whT_sb = cp.tile([128, 384], BF16)
            nc.sync.dma_start(out=whT_sb[:], in_=whT[:])
            id_sb = cp.tile([128, 128], BF16)
            nc.sync.dma_start(out=id_sb[:], in_=ident[:])
            gxb_sb = cp.tile([128, 3], F32)
            nc.sync.dma_start(out=gxb_sb[:], in_=gxb[:])
            bhn_sb = cp.tile([128, 1], F32)
            nc.sync.dma_start(out=bhn_sb[:], in_=bhn[:])

            # gx for r,z split into bf16 hi+lo (id-matmul'd into PSUM at
            # full precision); gx_n kept fp32 (added on DVE/GP side)
            gxhi = [cp.tile([128, T], BF16, name=f"gxhi{g}", tag=f"gxhi{g}")
                    for g in range(2)]
            gxlo = [cp.tile([128, T], BF16, name=f"gxlo{g}", tag=f"gxlo{g}")
                    for g in range(2)]
            gxn_sb = cp.tile([128, T], F32)
            hseq = cp.tile([128, T + 1], BF16)
            nc.vector.memset(hseq[:], 0.0)
            hfin = cp.tile([128, T], F32)

            # ---- gx build: gx_g = w_ih_g @ x + bias_g  (fp32 matmuls)
            with tc.tile_pool(name="gxp", bufs=2, space="PSUM") as gpp:
                for g in range(3):
                    for j in range(NCH):
                        c0 = j * CH
                        psg = gpp.tile([128, CH], F32, tag="gps")
                        nc.tensor.matmul(
                            out=psg[:],
                            lhsT=wiT_sb[:, g * 128:(g + 1) * 128],
                            rhs=x_sb[:, c0:c0 + CH],
                            start=True, stop=True)
                        if g < 2:
                            nc.vector.tensor_scalar_add(
                                out=gxhi[g][:, c0:c0 + CH], in0=psg[:],
                                scalar1=gxb_sb[:, g:g + 1])
                            # lo = (psg + bias) - hi   (residual, bf16)
                            nc.vector.scalar_tensor_tensor(
                                out=gxlo[g][:, c0:c0 + CH], in0=psg[:],
                                scalar=gxb_sb[:, g:g + 1],
                                in1=gxhi[g][:, c0:c0 + CH],
                                op0=ADD, op1=SUB)
                        else:
                            nc.vector.tensor_scalar_add(
                                out=gxn_sb[:, c0:c0 + CH], in0=psg[:],
                                scalar1=gxb_sb[:, g:g + 1])

            # ---- DEER iterations
            for k in range(iters):
                last = k == iters - 1
                for j in range(NCH):
                    c0 = j * CH
                    ps_rz = pp.tile([128, 2, CH], F32, tag="psrz")
                    ps_n = pp.tile([128, CH], F32, tag="psn")
                    rhs_h = hseq[:, c0:c0 + CH]
                    # gx preload via identity matmuls (hi+lo), then W_hh@h
                    nc.tensor.matmul(out=ps_rz[:, 0, :], lhsT=id_sb[:],
                                     rhs=gxhi[0][:, c0:c0 + CH],
                                     start=True, stop=False)
                    nc.tensor.matmul(out=ps_rz[:, 0, :], lhsT=id_sb[:],
                                     rhs=gxlo[0][:, c0:c0 + CH],
                                     start=False, stop=False)
                    nc.tensor.matmul(out=ps_rz[:, 1, :], lhsT=id_sb[:],
                                     rhs=gxhi[1][:, c0:c0 + CH],
                                     start=True, stop=False)
                    nc.tensor.matmul(out=ps_rz[:, 1, :], lhsT=id_sb[:],
                                     rhs=gxlo[1][:, c0:c0 + CH],
                                     start=False, stop=False)
                    nc.tensor.matmul(out=ps_rz[:, 0, :],
                                     lhsT=whT_sb[:, 0:128],
                                     rhs=rhs_h, start=False, stop=False)
                    nc.tensor.matmul(out=ps_rz[:, 1, :],
                                     lhsT=whT_sb[:, 128:256],
                                     rhs=rhs_h, start=False, stop=True)
                    nc.tensor.matmul(out=ps_n[:],
                                     lhsT=whT_sb[:, 256:384],
                                     rhs=rhs_h, start=True, stop=True)

                    rz = sp.tile([128, 2, CH], F32, tag="rz")
                    nc.scalar.activation(rz[:], ps_rz[:], SIG)
                    # u = r * (gh_n + b_hh_n)
                    u = sp.tile([128, CH], F32, tag="u")
                    nc.vector.scalar_tensor_tensor(
                        out=u[:], in0=ps_n[:], scalar=bhn_sb[:, 0:1],
                        in1=rz[:, 0, :], op0=ADD, op1=MULT)
                    # v = u + gx_n
                    v = sp.tile([128, CH], F32, tag="v")
                    nc.gpsimd.tensor_tensor(
                        out=v[:], in0=u[:], in1=gxn_sb[:, c0:c0 + CH],
                        op=ADD)
                    n_t = sp.tile([128, CH], F32, tag="n")
                    nc.scalar.activation(n_t[:], v[:], TANH)
                    # w = (1 - z) * n
                    zp = sp.tile([128, CH], F32, tag="zp")
                    nc.gpsimd.tensor_scalar(
                        out=zp[:], in0=rz[:, 1, :], scalar1=-1.0,
                        scalar2=1.0, op0=MULT, op1=ADD)
                    w_t = sp.tile([128, CH], F32, tag="w")
                    nc.vector.tensor_tensor(out=w_t[:], in0=zp[:],
                                            in1=n_t[:], op=MULT)
                    # h_t = z_t * h_{t-1} + w_t over this chunk
                    if last:
                        nc.vector.tensor_tensor_scan(
                            out=hfin[:, c0:c0 + CH],
                            data0=rz[:, 1, :], data1=w_t[:],
                            initial=(0.0 if j == 0
                                     else hfin[:, c0 - 1:c0]),
                            op0=MULT, op1=ADD)
                    else:
                        nc.vector.tensor_tensor_scan(
                            out=hseq[:, c0 + 1:c0 + CH + 1],
                            data0=rz[:, 1, :], data1=w_t[:],
                            initial=(0.0 if j == 0
                                     else hseq[:, c0:c0 + 1]),
                            op0=MULT, op1=ADD)

            hm4 = cp.tile([128, NCH], F32)
            for j in range(NCH):
                nc.vector.tensor_reduce(
                    out=hm4[:, j:j + 1], in_=hfin[:, j * CH:(j + 1) * CH],
                    axis=mybir.AxisListType.X, op=mybir.AluOpType.max)
            hm = cp.tile([128, 1], F32)
            nc.vector.tensor_reduce(out=hm[:], in_=hm4[:],
                                    axis=mybir.AxisListType.X,
                                    op=mybir.AluOpType.max)
            nc.sync.dma_start(out=hmax[:], in_=hm[:])
    nc.finalize()
    return nc


_PROGS = {}


def _get(name, builder):
    if name not in _PROGS:
        _PROGS[name] = builder()
    return _PROGS[name]


# ---------------------------------------------------------------- driver
def kernel(tokens, parent, depth, tree_id, emb, Wc, bc,
           w_ih_f, w_hh_f, b_ih_f, b_hh_f,
           w_ih_b, w_hh_b, b_ih_b, b_hh_b, T):
    tokens = np.asarray(tokens).astype(np.int32)
    emb = np.asarray(emb, dtype=np.float32)
    Wc = np.asarray(Wc, dtype=np.float32)
    bc = np.asarray(bc, dtype=np.float32)
    LAST_RESULTS.clear()

    # ---- host: projected embedding table (weights-only constant fold)
    TW = (emb @ Wc.T + bc).astype(ml_dtypes.bfloat16)

    # ---- K1: tree encodings, tree-sharded
    nc1, S = _get("k1", build_k1)
    S00T = np.ascontiguousarray(S[0:128, 0:128].T).astype(ml_dtypes.bfloat16)
    RHI = np.ascontiguousarray(
        np.concatenate([S[0:128, 128:256].T, np.eye(128, dtype=np.float32)],
                       axis=1)).astype(ml_dtypes.bfloat16)
    in1 = []
    for i in range(NCORES):
        tk = tokens[i * NODES_PER_CORE:(i + 1) * NODES_PER_CORE]
        # dma_gather idx wrap: idx[16k+i, s] = tokens[s*16+i], k=0..7
        wrap = np.ascontiguousarray(tk.reshape(-1, 16).T.astype(np.int16))
        idx = np.ascontiguousarray(np.tile(wrap, (8, 1)))   # [128, 4096]
        in1.append({"tw": TW, "idx": idx, "s00t": S00T, "rhi": RHI})
    r1 = run_bass_kernel_spmd(nc1, in1, core_ids=list(range(NCORES)),
                              **_TRACE_KW)
    LAST_RESULTS.append(r1)
    te = np.concatenate([r1.results[i]["te"] for i in range(NCORES)],
                        axis=1)                              # [128, 2048]

    # ---- K2: DEER GRU fwd (core 0) + bwd (core 1)
    nc2 = _get("k2", build_k2)
    ident = np.eye(128, dtype=np.float32).astype(ml_dtypes.bfloat16)

    def gru_inputs(x_seq, w_ih, w_hh, b_ih, b_hh):
        w_ih = np.asarray(w_ih, np.float32)
        w_hh = np.asarray(w_hh, np.float32)
        b_ih = np.asarray(b_ih, np.float32)
        b_hh = np.asarray(b_hh, np.float32)
        wiT = np.concatenate(
            [np.ascontiguousarray(w_ih[g * H:(g + 1) * H].T)
             for g in range(3)], axis=1)
        whT = np.concatenate(
            [np.ascontiguousarray(w_hh[g * H:(g + 1) * H].T)
             for g in range(3)], axis=1).astype(ml_dtypes.bfloat16)
        gxb = np.stack([
            b_ih[0:128] + b_hh[0:128],
            b_ih[128:256] + b_hh[128:256],
            b_ih[256:384],
        ], axis=1).astype(np.float32)
        return {"x": np.ascontiguousarray(x_seq, np.float32), "wiT": wiT,
                "whT": whT, "ident": ident, "gxb": gxb,
                "bhn": np.ascontiguousarray(b_hh[256:384].reshape(128, 1))}

    in2 = [
        gru_inputs(te, w_ih_f, w_hh_f, b_ih_f, b_hh_f),
        gru_inputs(te[:, ::-1], w_ih_b, w_hh_b, b_ih_b, b_hh_b),
    ]
    r2 = run_bass_kernel_spmd(nc2, in2, core_ids=[0, 1], **_TRACE_KW)
    LAST_RESULTS.append(r2)
    fwd_max = r2.results[0]["hmax"][:, 0]
    bwd_max = r2.results[1]["hmax"][:, 0]
    return np.concatenate([fwd_max, bwd_max]).astype(np.float32)



# revision 3
# speedup vs baseline: 2.5833x; 2.5833x over previous
"""Trainium2 Bass kernel for nn_BatchProgramCC (gnn_message_passing).

Pipeline (2 NEFF launches):
  Host:  TW = emb @ Wc.T + bc  (weight-only constant fold), cast bf16.
  K1 (8 cores, tree-sharded): batched SWDGE dma_gather of TW rows by
      token, split into 32 gathers of 2048 rows round-robin over 4 SWDGE
      queues (descriptor generation on gpsimd is the bottleneck and
      parallelizes per queue).  Per-tree subtree sums via bf16 structure
      matmuls (output transposed to [ch, node]), per-tree max on DVE
      (4 trees per reduce from 2-bank PSUM), relu -> te shard.
  K2 (2 cores: fwd / bwd): parallel-in-time GRU via DEER fixed-point
      iteration.  Given gates, the h-recurrence is linear-diagonal:
      h_t = z_t*h_{t-1} + (1-z_t)*n_t, evaluated with the hardware
      tensor_tensor_scan (fp32 internal state).  Gates are recomputed
      from the previous iterate's h-sequence with bf16 matmuls; gx is
      cached fp32 in SBUF and added on DVE (no PSUM identity preloads).
      4 iterations converge to ~1.5e-3 vs the 2e-2 gate.  Final max
      over t on DVE.

Self-contained: hardcodes all shapes; no sibling imports.
"""

import numpy as np
import ml_dtypes

import concourse.bass as bass
import concourse.mybir as mybir
from concourse import bacc
from concourse.tile import TileContext
from concourse.bass_utils import run_bass_kernel_spmd

F32 = mybir.dt.float32
BF16 = mybir.dt.bfloat16
I16 = mybir.dt.int16
I32 = mybir.dt.int32

T_TREES = 2048
P = 256          # nodes per tree
KARY = 4
VOCAB = 30000
E = 128
C = 128
H = 128
NCORES = 8
TREES_PER_CORE = T_TREES // NCORES          # 256
NODES_PER_CORE = TREES_PER_CORE * P         # 65536

DEER_ITERS = 4
NQUEUES = 4          # SWDGE queues for the K1 gather
GROWS = 2048         # rows per dma_gather (8 trees)

LAST_RESULTS = []   # BassKernelResults stash for test.py profiling
_TRACE_KW = {}      # test.py may set {'trace': True}


def _tree_struct():
    """S[i, j] = 1 iff node j is in subtree(i) (including i==j)."""
    pl = np.zeros(P, np.int64)
    for i in range(1, P):
        pl[i] = (i - 1) // KARY
    S = np.zeros((P, P), np.float32)
    for j in range(P):
        a = j
        while True:
            S[a, j] = 1.0
            if a == 0:
                break
            a = int(pl[a])
    return S


# ---------------------------------------------------------------- K1: trees
def build_k1():
    S = _tree_struct()
    nc = bacc.Bacc("TRN2", target_bir_lowering=False, debug=False,
                   num_devices=NCORES, num_swdge_queues=NQUEUES)
    tw = nc.dram_tensor("tw", [VOCAB, C], BF16, kind="ExternalInput")
    idx = nc.dram_tensor("idx", [128, NODES_PER_CORE // 16], I16,
                         kind="ExternalInput")
    s00t = nc.dram_tensor("s00t", [128, 128], BF16, kind="ExternalInput")
    rhi = nc.dram_tensor("rhi", [128, 256], BF16, kind="ExternalInput")
    te = nc.dram_tensor("te", [128, TREES_PER_CORE], F32,
                        kind="ExternalOutput")

    NIDX = NODES_PER_CORE // 16      # 4096 idx columns (16-partition wrap)
    NGATHER = NODES_PER_CORE // GROWS   # 32
    GIDXC = GROWS // 16              # idx columns per gather
    TREES_PER_GATHER = GROWS // P    # 8

    with TileContext(nc) as tc:
        with (
            tc.tile_pool(name="const", bufs=1) as cp,
            tc.tile_pool(name="gat", bufs=8) as gp,
            tc.tile_pool(name="psum", bufs=3, space="PSUM") as pp,
        ):
            idx_sb = cp.tile([128, NIDX], I16)
            nc.sync.dma_start(out=idx_sb[:], in_=idx[:])
            s00t_sb = cp.tile([128, 128], BF16)
            nc.sync.dma_start(out=s00t_sb[:], in_=s00t[:])
            rhi_sb = cp.tile([128, 256], BF16)
            nc.sync.dma_start(out=rhi_sb[:], in_=rhi[:])
            te_sb = cp.tile([128, TREES_PER_CORE], F32)

            for g in range(NGATHER):
                gat = gp.tile([128, GROWS // 128, C], BF16, tag="gat")
                nc.gpsimd.dma_gather(
                    gat[:], tw[:],
                    idx_sb[:, g * GIDXC:(g + 1) * GIDXC],
                    GROWS, GROWS, C, single_packet=False,
                    queue_num=g % NQUEUES)
                # 8 trees; 4 trees share one 2-bank psum tile
                for q in range(TREES_PER_GATHER // 4):
                    ps = pp.tile([128, 4, 256], F32, tag="ps")
                    for ti in range(4):
                        t_in_tile = q * 4 + ti
                        lo = gat[:, 2 * t_in_tile, :]
                        hi = gat[:, 2 * t_in_tile + 1, :]
                        nc.tensor.matmul(out=ps[:, ti, :], lhsT=hi,
                                         rhs=rhi_sb[:], start=True,
                                         stop=False)
                        nc.tensor.matmul(out=ps[:, ti, 0:128], lhsT=lo,
                                         rhs=s00t_sb[:], start=False,
                                         stop=True)
                    t0 = g * TREES_PER_GATHER + q * 4
                    nc.vector.tensor_reduce(
                        out=te_sb[:, t0:t0 + 4], in_=ps[:],
                        axis=mybir.AxisListType.X, op=mybir.AluOpType.max)
            nc.vector.tensor_scalar_max(out=te_sb[:], in0=te_sb[:],
                                        scalar1=0.0)
            nc.sync.dma_start(out=te[:], in_=te_sb[:])
    nc.finalize()
    return nc, S


# ---------------------------------------------------------------- K2: GRU
def build_k2(iters=None):
    iters = iters or DEER_ITERS
    T = T_TREES
    nc = bacc.Bacc("TRN2", target_bir_lowering=False, debug=False,
                   num_devices=2)
    x = nc.dram_tensor("x", [128, T], F32, kind="ExternalInput")
    wiT = nc.dram_tensor("wiT", [128, 384], F32, kind="ExternalInput")
    whT = nc.dram_tensor("whT", [128, 384], BF16, kind="ExternalInput")
    gxb = nc.dram_tensor("gxb", [128, 3], F32, kind="ExternalInput")
    bhn = nc.dram_tensor("bhn", [128, 1], F32, kind="ExternalInput")
    hmax = nc.dram_tensor("hmax", [128, 1], F32, kind="ExternalOutput")

    SIG = mybir.ActivationFunctionType.Sigmoid
    TANH = mybir.ActivationFunctionType.Tanh
    MULT = mybir.AluOpType.mult
    ADD = mybir.AluOpType.add
    SUB = mybir.AluOpType.subtract

    CH = 512                 # column chunk
    NCH = T // CH            # 4

    with TileContext(nc) as tc:
        with (
            tc.tile_pool(name="const", bufs=1) as cp,
            tc.tile_pool(name="step", bufs=4) as sp,
            tc.tile_pool(name="psum", bufs=2, space="PSUM") as pp,
        ):
            x_sb = cp.tile([128, T], F32)
            nc.sync.dma_start(out=x_sb[:], in_=x[:])
            wiT_sb = cp.tile([128, 384], F32)
            nc.sync.dma_start(out=wiT_sb[:], in_=wiT[:])
            whT_sb = cp.tile([128, 384], BF16)
            nc.sync.dma_start(out=whT_sb[:], in_=whT[:])
            gxb_sb = cp.tile([128, 3], F32)
            nc.sync.dma_start(out=gxb_sb[:], in_=gxb[:])
            bhn_sb = cp.tile([128, 1], F32)
            nc.sync.dma_start(out=bhn_sb[:], in_=bhn[:])

            gx_sb = cp.tile([128, 3, T], F32)      # r,z,n gate preacts
            hseq = cp.tile([128, T + 1], BF16)
            nc.vector.memset(hseq[:], 0.0)
            hfin = cp.tile([128, T], F32)

            # ---- gx build: gx_g = w_ih_g @ x + bias_g  (fp32 matmuls)
            with tc.tile_pool(name="gxp", bufs=2, space="PSUM") as gpp:
                for g in range(3):
                    for j in range(NCH):
                        c0 = j * CH
                        psg = gpp.tile([128, CH], F32, tag="gps")
                        nc.tensor.matmul(
                            out=psg[:],
                            lhsT=wiT_sb[:, g * 128:(g + 1) * 128],
                            rhs=x_sb[:, c0:c0 + CH],
                            start=True, stop=True)
                        nc.vector.tensor_scalar_add(
                            out=gx_sb[:, g, c0:c0 + CH], in0=psg[:],
                            scalar1=gxb_sb[:, g:g + 1])

            # ---- DEER iterations
            for k in range(iters):
                last = k == iters - 1
                for j in range(NCH):
                    c0 = j * CH
                    ps_rz = pp.tile([128, 2, CH], F32, tag="psrz")
                    ps_n = pp.tile([128, CH], F32, tag="psn")
                    rhs_h = hseq[:, c0:c0 + CH]
                    nc.tensor.matmul(out=ps_rz[:, 0, :],
                                     lhsT=whT_sb[:, 0:128],
                                     rhs=rhs_h, start=True, stop=True)
                    nc.tensor.matmul(out=ps_rz[:, 1, :],
                                     lhsT=whT_sb[:, 128:256],
                                     rhs=rhs_h, start=True, stop=True)
                    nc.tensor.matmul(out=ps_n[:],
                                     lhsT=whT_sb[:, 256:384],
                                     rhs=rhs_h, start=True, stop=True)

                    # gate preacts = gh + gx  (DVE, bf16 out)
                    rz_in = sp.tile([128, 2, CH], BF16, tag="rzin")
                    nc.vector.tensor_tensor(
                        out=rz_in[:], in0=ps_rz[:],
                        in1=gx_sb[:, 0:2, c0:c0 + CH], op=ADD)
                    rz = sp.tile([128, 2, CH], F32, tag="rz")
                    nc.scalar.activation(rz[:], rz_in[:], SIG)
                    # u = r * (gh_n + b_hh_n)
                    u = sp.tile([128, CH], F32, tag="u")
                    nc.vector.scalar_tensor_tensor(
                        out=u[:], in0=ps_n[:], scalar=bhn_sb[:, 0:1],
                        in1=rz[:, 0, :], op0=ADD, op1=MULT)
                    # v = u + gx_n
                    v = sp.tile([128, CH], F32, tag="v")
                    nc.gpsimd.tensor_tensor(
                        out=v[:], in0=u[:], in1=gx_sb[:, 2, c0:c0 + CH],
                        op=ADD)
                    n_t = sp.tile([128, CH], F32, tag="n")
                    nc.scalar.activation(n_t[:], v[:], TANH)
                    # negw = (z - 1) * n  == -(1-z)*n
                    negw = sp.tile([128, CH], F32, tag="negw")
                    nc.vector.scalar_tensor_tensor(
                        out=negw[:], in0=rz[:, 1, :], scalar=1.0,
                        in1=n_t[:], op0=SUB, op1=MULT)
                    # h_t = z_t * h_{t-1} - negw_t over this chunk
                    if last:
                        nc.vector.tensor_tensor_scan(
                            out=hfin[:, c0:c0 + CH],
                            data0=rz[:, 1, :], data1=negw[:],
                            initial=(0.0 if j == 0
                                     else hfin[:, c0 - 1:c0]),
                            op0=MULT, op1=SUB)
                    else:
                        nc.vector.tensor_tensor_scan(
                            out=hseq[:, c0 + 1:c0 + CH + 1],
                            data0=rz[:, 1, :], data1=negw[:],
                            initial=(0.0 if j == 0
                                     else hseq[:, c0:c0 + 1]),
                            op0=MULT, op1=SUB)

            hm4 = cp.tile([128, NCH], F32)
            for j in range(NCH):
                nc.vector.tensor_reduce(
                    out=hm4[:, j:j + 1], in_=hfin[:, j * CH:(j + 1) * CH],
                    axis=mybir.AxisListType.X, op=mybir.AluOpType.max)
            hm = cp.tile([128, 1], F32)
            nc.vector.tensor_reduce(out=hm[:], in_=hm4[:],
                                    axis=mybir.AxisListType.X,
                                    op=mybir.AluOpType.max)
            nc.sync.dma_start(out=hmax[:], in_=hm[:])
    nc.finalize()
    return nc


_PROGS = {}


def _get(name, builder):
    if name not in _PROGS:
        _PROGS[name] = builder()
    return _PROGS[name]


# ---------------------------------------------------------------- driver
def kernel(tokens, parent, depth, tree_id, emb, Wc, bc,
           w_ih_f, w_hh_f, b_ih_f, b_hh_f,
           w_ih_b, w_hh_b, b_ih_b, b_hh_b, T):
    tokens = np.asarray(tokens).astype(np.int32)
    emb = np.asarray(emb, dtype=np.float32)
    Wc = np.asarray(Wc, dtype=np.float32)
    bc = np.asarray(bc, dtype=np.float32)
    LAST_RESULTS.clear()

    # ---- host: projected embedding table (weights-only constant fold)
    TW = (emb @ Wc.T + bc).astype(ml_dtypes.bfloat16)

    # ---- K1: tree encodings, tree-sharded
    nc1, S = _get("k1", build_k1)
    S00T = np.ascontiguousarray(S[0:128, 0:128].T).astype(ml_dtypes.bfloat16)
    RHI = np.ascontiguousarray(
        np.concatenate([S[0:128, 128:256].T, np.eye(128, dtype=np.float32)],
                       axis=1)).astype(ml_dtypes.bfloat16)
    in1 = []
    for i in range(NCORES):
        tk = tokens[i * NODES_PER_CORE:(i + 1) * NODES_PER_CORE]
        # dma_gather idx wrap: idx[16k+i, s] = tokens[s*16+i], k=0..7
        wrap = np.ascontiguousarray(tk.reshape(-1, 16).T.astype(np.int16))
        idx = np.ascontiguousarray(np.tile(wrap, (8, 1)))   # [128, 4096]
        in1.append({"tw": TW, "idx": idx, "s00t": S00T, "rhi": RHI})
    r1 = run_bass_kernel_spmd(nc1, in1, core_ids=list(range(NCORES)),
                              **_TRACE_KW)
    LAST_RESULTS.append(r1)
    te = np.concatenate([r1.results[i]["te"] for i in range(NCORES)],
                        axis=1)                              # [128, 2048]

    # ---- K2: DEER GRU fwd (core 0) + bwd (core 1)
    nc2 = _get("k2", build_k2)

    def gru_inputs(x_seq, w_ih, w_hh, b_ih, b_hh):
        w_ih = np.asarray(w_ih, np.float32)
        w_hh = np.asarray(w_hh, np.float32)
        b_ih = np.asarray(b_ih, np.float32)
        b_hh = np.asarray(b_hh, np.float32)
        wiT = np.concatenate(
            [np.ascontiguousarray(w_ih[g * H:(g + 1) * H].T)
             for g in range(3)], axis=1)
        whT = np.concatenate(
            [np.ascontiguousarray(w_hh[g * H:(g + 1) * H].T)
             for g in range(3)], axis=1).astype(ml_dtypes.bfloat16)
        gxb = np.stack([
            b_ih[0:128] + b_hh[0:128],
            b_ih[128:256] + b_hh[128:256],
            b_ih[256:384],
        ], axis=1).astype(np.float32)
        return {"x": np.ascontiguousarray(x_seq, np.float32), "wiT": wiT,
                "whT": whT, "gxb": gxb,
                "bhn": np.ascontiguousarray(b_hh[256:384].reshape(128, 1))}

    in2 = [
        gru_inputs(te, w_ih_f, w_hh_f, b_ih_f, b_hh_f),
        gru_inputs(te[:, ::-1], w_ih_b, w_hh_b, b_ih_b, b_hh_b),
    ]
    r2 = run_bass_kernel_spmd(nc2, in2, core_ids=[0, 1], **_TRACE_KW)
    LAST_RESULTS.append(r2)
    fwd_max = r2.results[0]["hmax"][:, 0]
    bwd_max = r2.results[1]["hmax"][:, 0]
    return np.concatenate([fwd_max, bwd_max]).astype(np.float32)


# revision 8
# speedup vs baseline: 2.9096x; 1.1263x over previous
"""Trainium2 Bass kernel for nn_BatchProgramCC (gnn_message_passing).

Pipeline (2 NEFF launches):
  Host:  TW = emb @ Wc.T + bc  (weight-only constant fold), cast bf16.
  K1 (8 cores, tree-sharded): batched SWDGE dma_gather of TW rows by
      token, split into 32 gathers of 2048 rows round-robin over 4 SWDGE
      queues (descriptor generation on gpsimd is the bottleneck and
      parallelizes per queue).  Per-tree subtree sums via bf16 structure
      matmuls (output transposed to [ch, node]), per-tree max on DVE
      (4 trees per reduce from 2-bank PSUM), relu -> te shard.
  K2 (2 cores: fwd / bwd): parallel-in-time GRU via DEER fixed-point
      iteration.  Given gates, the h-recurrence is linear-diagonal:
      h_t = z_t*h_{t-1} + (1-z_t)*n_t, evaluated with the hardware
      tensor_tensor_scan (fp32 internal state).  Gates are recomputed
      from the previous iterate's h-sequence with bf16 matmuls; gx is
      cached fp32 in SBUF and added on DVE (no PSUM identity preloads).
      4 iterations converge to ~1.5e-3 vs the 2e-2 gate.  Final max
      over t on DVE.

Self-contained: hardcodes all shapes; no sibling imports.
"""

import numpy as np
import ml_dtypes

import concourse.bass as bass
import concourse.mybir as mybir
from concourse import bacc
from concourse.tile import TileContext
from concourse.bass_utils import run_bass_kernel_spmd

F32 = mybir.dt.float32
BF16 = mybir.dt.bfloat16
I16 = mybir.dt.int16
I32 = mybir.dt.int32

T_TREES = 2048
P = 256          # nodes per tree
KARY = 4
VOCAB = 30000
E = 128
C = 128
H = 128
NCORES = 8
TREES_PER_CORE = T_TREES // NCORES          # 256
NODES_PER_CORE = TREES_PER_CORE * P         # 65536

DEER_ITERS = 4
NQUEUES = 4          # SWDGE queues for the K1 gather
GROWS = 2048         # rows per dma_gather (8 trees)

LAST_RESULTS = []   # BassKernelResults stash for test.py profiling
_TRACE_KW = {}      # test.py may set {'trace': True}


def _tree_struct():
    """S[i, j] = 1 iff node j is in subtree(i) (including i==j)."""
    pl = np.zeros(P, np.int64)
    for i in range(1, P):
        pl[i] = (i - 1) // KARY
    S = np.zeros((P, P), np.float32)
    for j in range(P):
        a = j
        while True:
            S[a, j] = 1.0
            if a == 0:
                break
            a = int(pl[a])
    return S


# ---------------------------------------------------------------- K1: trees
def build_k1():
    S = _tree_struct()
    nc = bacc.Bacc("TRN2", target_bir_lowering=False, debug=False,
                   num_devices=NCORES, num_swdge_queues=NQUEUES)
    tw = nc.dram_tensor("tw", [VOCAB, C], BF16, kind="ExternalInput")
    idx = nc.dram_tensor("idx", [128, NODES_PER_CORE // 16], I16,
                         kind="ExternalInput")
    s00t = nc.dram_tensor("s00t", [128, 128], BF16, kind="ExternalInput")
    rhi = nc.dram_tensor("rhi", [128, 256], BF16, kind="ExternalInput")
    te = nc.dram_tensor("te", [128, TREES_PER_CORE], F32,
                        kind="ExternalOutput")

    NIDX = NODES_PER_CORE // 16      # 4096 idx columns (16-partition wrap)
    NGATHER = NODES_PER_CORE // GROWS   # 32
    GIDXC = GROWS // 16              # idx columns per gather
    TREES_PER_GATHER = GROWS // P    # 8

    with TileContext(nc) as tc:
        with (
            tc.tile_pool(name="const", bufs=1) as cp,
            tc.tile_pool(name="gat", bufs=8) as gp,
            tc.tile_pool(name="psum", bufs=3, space="PSUM") as pp,
        ):
            idx_sb = cp.tile([128, NIDX], I16)
            nc.sync.dma_start(out=idx_sb[:], in_=idx[:])
            s00t_sb = cp.tile([128, 128], BF16)
            nc.sync.dma_start(out=s00t_sb[:], in_=s00t[:])
            rhi_sb = cp.tile([128, 256], BF16)
            nc.sync.dma_start(out=rhi_sb[:], in_=rhi[:])
            te_sb = cp.tile([128, TREES_PER_CORE], F32)

            for g in range(NGATHER):
                gat = gp.tile([128, GROWS // 128, C], BF16, tag="gat")
                nc.gpsimd.dma_gather(
                    gat[:], tw[:],
                    idx_sb[:, g * GIDXC:(g + 1) * GIDXC],
                    GROWS, GROWS, C, single_packet=False,
                    queue_num=g % NQUEUES)
                # 8 trees; 4 trees share one 2-bank psum tile
                for q in range(TREES_PER_GATHER // 4):
                    ps = pp.tile([128, 4, 256], F32, tag="ps")
                    for ti in range(4):
                        t_in_tile = q * 4 + ti
                        lo = gat[:, 2 * t_in_tile, :]
                        hi = gat[:, 2 * t_in_tile + 1, :]
                        nc.tensor.matmul(out=ps[:, ti, :], lhsT=hi,
                                         rhs=rhi_sb[:], start=True,
                                         stop=False)
                        nc.tensor.matmul(out=ps[:, ti, 0:128], lhsT=lo,
                                         rhs=s00t_sb[:], start=False,
                                         stop=True)
                    t0 = g * TREES_PER_GATHER + q * 4
                    nc.vector.tensor_reduce(
                        out=te_sb[:, t0:t0 + 4], in_=ps[:],
                        axis=mybir.AxisListType.X, op=mybir.AluOpType.max)
            nc.vector.tensor_scalar_max(out=te_sb[:], in0=te_sb[:],
                                        scalar1=0.0)
            nc.sync.dma_start(out=te[:], in_=te_sb[:])
    nc.finalize()
    return nc, S


# ---------------------------------------------------------------- K2: GRU
def build_k2(iters=None):
    iters = iters or DEER_ITERS
    T = T_TREES
    nc = bacc.Bacc("TRN2", target_bir_lowering=False, debug=False,
                   num_devices=2)
    x = nc.dram_tensor("x", [128, T], F32, kind="ExternalInput")
    wiT = nc.dram_tensor("wiT", [128, 384], F32, kind="ExternalInput")
    whT = nc.dram_tensor("whT", [128, 384], BF16, kind="ExternalInput")
    gxb = nc.dram_tensor("gxb", [128, 3], F32, kind="ExternalInput")
    bhn = nc.dram_tensor("bhn", [128, 1], F32, kind="ExternalInput")
    hmax = nc.dram_tensor("hmax", [128, 1], F32, kind="ExternalOutput")

    SIG = mybir.ActivationFunctionType.Sigmoid
    TANH = mybir.ActivationFunctionType.Tanh
    MULT = mybir.AluOpType.mult
    ADD = mybir.AluOpType.add
    SUB = mybir.AluOpType.subtract

    CH = 512                 # column chunk
    NCH = T // CH            # 4

    with TileContext(nc) as tc:
        with (
            tc.tile_pool(name="const", bufs=1) as cp,
            tc.tile_pool(name="step", bufs=4) as sp,
            tc.tile_pool(name="psum", bufs=2, space="PSUM") as pp,
        ):
            x_sb = cp.tile([128, T], F32)
            nc.sync.dma_start(out=x_sb[:], in_=x[:])
            wiT_sb = cp.tile([128, 384], F32)
            nc.sync.dma_start(out=wiT_sb[:], in_=wiT[:])
            whT_sb = cp.tile([128, 384], BF16)
            nc.sync.dma_start(out=whT_sb[:], in_=whT[:])
            gxb_sb = cp.tile([128, 3], F32)
            nc.sync.dma_start(out=gxb_sb[:], in_=gxb[:])
            bhn_sb = cp.tile([128, 1], F32)
            nc.sync.dma_start(out=bhn_sb[:], in_=bhn[:])

            gx_sb = cp.tile([128, 3, T], F32)      # r,z,n gate preacts
            # ping-pong h-sequence buffers: iteration k reads hs[k%2],
            # writes hs[1-k%2] (pure Jacobi DEER) so chunk j+1's gate
            # matmuls don't serialize behind chunk j's scan.
            hs = [cp.tile([128, T + 1], BF16, name=f"hseq{i}", tag=f"hseq{i}")
                  for i in range(2)]
            nc.vector.memset(hs[0][:], 0.0)
            nc.vector.memset(hs[1][:, 0:1], 0.0)
            hfin = cp.tile([128, T], F32)

            # ---- gx build: gx_g = w_ih_g @ x + bias_g  (fp32 matmuls)
            with tc.tile_pool(name="gxp", bufs=2, space="PSUM") as gpp:
                for g in range(3):
                    for j in range(NCH):
                        c0 = j * CH
                        psg = gpp.tile([128, CH], F32, tag="gps")
                        nc.tensor.matmul(
                            out=psg[:],
                            lhsT=wiT_sb[:, g * 128:(g + 1) * 128],
                            rhs=x_sb[:, c0:c0 + CH],
                            start=True, stop=True)
                        nc.vector.tensor_scalar_add(
                            out=gx_sb[:, g, c0:c0 + CH], in0=psg[:],
                            scalar1=gxb_sb[:, g:g + 1])

            # ---- DEER iterations
            for k in range(iters):
                last = k == iters - 1
                hseq = hs[k % 2]          # read buffer (h from iter k-1)
                hnxt = hs[1 - k % 2]      # write buffer (this iter's h)
                for j in range(NCH):
                    c0 = j * CH
                    ps_rz = pp.tile([128, 2, CH], F32, tag="psrz")
                    ps_n = pp.tile([128, CH], F32, tag="psn")
                    rhs_h = hseq[:, c0:c0 + CH]
                    nc.tensor.matmul(out=ps_rz[:, 0, :],
                                     lhsT=whT_sb[:, 0:128],
                                     rhs=rhs_h, start=True, stop=True)
                    nc.tensor.matmul(out=ps_rz[:, 1, :],
                                     lhsT=whT_sb[:, 128:256],
                                     rhs=rhs_h, start=True, stop=True)
                    nc.tensor.matmul(out=ps_n[:],
                                     lhsT=whT_sb[:, 256:384],
                                     rhs=rhs_h, start=True, stop=True)

                    # gate preacts = gh + gx  (DVE, bf16 out)
                    rz_in = sp.tile([128, 2, CH], BF16, tag="rzin")
                    nc.vector.tensor_tensor(
                        out=rz_in[:], in0=ps_rz[:],
                        in1=gx_sb[:, 0:2, c0:c0 + CH], op=ADD)
                    rz = sp.tile([128, 2, CH], F32, tag="rz")
                    nc.scalar.activation(rz[:], rz_in[:], SIG)
                    # u = r * (gh_n + b_hh_n)
                    u = sp.tile([128, CH], F32, tag="u")
                    nc.vector.scalar_tensor_tensor(
                        out=u[:], in0=ps_n[:], scalar=bhn_sb[:, 0:1],
                        in1=rz[:, 0, :], op0=ADD, op1=MULT)
                    # v = u + gx_n
                    v = sp.tile([128, CH], F32, tag="v")
                    nc.gpsimd.tensor_tensor(
                        out=v[:], in0=u[:], in1=gx_sb[:, 2, c0:c0 + CH],
                        op=ADD)
                    n_t = sp.tile([128, CH], F32, tag="n")
                    nc.scalar.activation(n_t[:], v[:], TANH)
                    # negw = (z - 1) * n  == -(1-z)*n
                    negw = sp.tile([128, CH], F32, tag="negw")
                    nc.vector.scalar_tensor_tensor(
                        out=negw[:], in0=rz[:, 1, :], scalar=1.0,
                        in1=n_t[:], op0=SUB, op1=MULT)
                    # h_t = z_t * h_{t-1} - negw_t over this chunk
                    if last:
                        nc.vector.tensor_tensor_scan(
                            out=hfin[:, c0:c0 + CH],
                            data0=rz[:, 1, :], data1=negw[:],
                            initial=(0.0 if j == 0
                                     else hfin[:, c0 - 1:c0]),
                            op0=MULT, op1=SUB)
                    else:
                        nc.vector.tensor_tensor_scan(
                            out=hnxt[:, c0 + 1:c0 + CH + 1],
                            data0=rz[:, 1, :], data1=negw[:],
                            initial=(0.0 if j == 0
                                     else hnxt[:, c0:c0 + 1]),
                            op0=MULT, op1=SUB)

            hm4 = cp.tile([128, NCH], F32)
            for j in range(NCH):
                nc.vector.tensor_reduce(
                    out=hm4[:, j:j + 1], in_=hfin[:, j * CH:(j + 1) * CH],
                    axis=mybir.AxisListType.X, op=mybir.AluOpType.max)
            hm = cp.tile([128, 1], F32)
            nc.vector.tensor_reduce(out=hm[:], in_=hm4[:],
                                    axis=mybir.AxisListType.X,
                                    op=mybir.AluOpType.max)
            nc.sync.dma_start(out=hmax[:], in_=hm[:])
    nc.finalize()
    return nc


_PROGS = {}


def _get(name, builder):
    if name not in _PROGS:
        _PROGS[name] = builder()
    return _PROGS[name]


# ---------------------------------------------------------------- driver
def kernel(tokens, parent, depth, tree_id, emb, Wc, bc,
           w_ih_f, w_hh_f, b_ih_f, b_hh_f,
           w_ih_b, w_hh_b, b_ih_b, b_hh_b, T):
    tokens = np.asarray(tokens).astype(np.int32)
    emb = np.asarray(emb, dtype=np.float32)
    Wc = np.asarray(Wc, dtype=np.float32)
    bc = np.asarray(bc, dtype=np.float32)
    LAST_RESULTS.clear()

    # ---- host: projected embedding table (weights-only constant fold)
    TW = (emb @ Wc.T + bc).astype(ml_dtypes.bfloat16)

    # ---- K1: tree encodings, tree-sharded
    nc1, S = _get("k1", build_k1)
    S00T = np.ascontiguousarray(S[0:128, 0:128].T).astype(ml_dtypes.bfloat16)
    RHI = np.ascontiguousarray(
        np.concatenate([S[0:128, 128:256].T, np.eye(128, dtype=np.float32)],
                       axis=1)).astype(ml_dtypes.bfloat16)
    in1 = []
    for i in range(NCORES):
        tk = tokens[i * NODES_PER_CORE:(i + 1) * NODES_PER_CORE]
        # dma_gather idx wrap: idx[16k+i, s] = tokens[s*16+i], k=0..7
        wrap = np.ascontiguousarray(tk.reshape(-1, 16).T.astype(np.int16))
        idx = np.ascontiguousarray(np.tile(wrap, (8, 1)))   # [128, 4096]
        in1.append({"tw": TW, "idx": idx, "s00t": S00T, "rhi": RHI})
    r1 = run_bass_kernel_spmd(nc1, in1, core_ids=list(range(NCORES)),
                              **_TRACE_KW)
    LAST_RESULTS.append(r1)
    te = np.concatenate([r1.results[i]["te"] for i in range(NCORES)],
                        axis=1)                              # [128, 2048]

    # ---- K2: DEER GRU fwd (core 0) + bwd (core 1)
    nc2 = _get("k2", build_k2)

    def gru_inputs(x_seq, w_ih, w_hh, b_ih, b_hh):
        w_ih = np.asarray(w_ih, np.float32)
        w_hh = np.asarray(w_hh, np.float32)
        b_ih = np.asarray(b_ih, np.float32)
        b_hh = np.asarray(b_hh, np.float32)
        wiT = np.concatenate(
            [np.ascontiguousarray(w_ih[g * H:(g + 1) * H].T)
             for g in range(3)], axis=1)
        whT = np.concatenate(
            [np.ascontiguousarray(w_hh[g * H:(g + 1) * H].T)
             for g in range(3)], axis=1).astype(ml_dtypes.bfloat16)
        gxb = np.stack([
            b_ih[0:128] + b_hh[0:128],
            b_ih[128:256] + b_hh[128:256],
            b_ih[256:384],
        ], axis=1).astype(np.float32)
        return {"x": np.ascontiguousarray(x_seq, np.float32), "wiT": wiT,
                "whT": whT, "gxb": gxb,
                "bhn": np.ascontiguousarray(b_hh[256:384].reshape(128, 1))}

    in2 = [
        gru_inputs(te, w_ih_f, w_hh_f, b_ih_f, b_hh_f),
        gru_inputs(te[:, ::-1], w_ih_b, w_hh_b, b_ih_b, b_hh_b),
    ]
    r2 = run_bass_kernel_spmd(nc2, in2, core_ids=[0, 1], **_TRACE_KW)
    LAST_RESULTS.append(r2)
    fwd_max = r2.results[0]["hmax"][:, 0]
    bwd_max = r2.results[1]["hmax"][:, 0]
    return np.concatenate([fwd_max, bwd_max]).astype(np.float32)
